# revision 4
# baseline (speedup 1.0000x reference)
"""Bass/Trainium2 kernel for nn_Attention (B=8, N=1024, C=768, H=12 heads).

Strategy: data-parallel over batch — core b computes batch element b entirely
(no collectives). Per core:
  qkv^T = (qkv_w @ x^T) for Q,K (o on partitions)  [fp32r matmuls]
  V     = x @ Wv^T (natural layout)
  per head:  S = Q K^T   -> attn = exp(S*scale)/Z  (row softmax, no max-sub;
             |S*scale| <= ~6 so exp is safe in fp32)
             S^T = K Q^T -> P^T_raw = exp(S^T*scale); out^T_raw = V^T-style
             accumulation (lhsT=v, rhs=P^T_raw); fixed up by 1/Z per query.
  out = out^T_raw^T @ proj_w^T * r + proj_b  (r folded per-head before proj)
Outputs: out [1024,768] fp32, attn [12,1024,1024] fp32 per core.
"""
import sys
for _p in ('/opt/trn_rl_repo', '/root/.axon_site/_ro/trn_rl_repo'):
    if _p not in sys.path:
        sys.path.append(_p)

import numpy as np
import concourse.bass as bass
import concourse.mybir as mybir
import concourse.tile as tile
from concourse.masks import make_identity
from concourse.bass_utils import run_bass_kernel_spmd

F32 = mybir.dt.float32
F32R = mybir.dt.float32r
AF = mybir.ActivationFunctionType
MULT = mybir.AluOpType.mult

P = 128
N = 1024          # tokens
NT = N // P       # 8
C = 768           # channels
CT = C // P       # 6
H = 12            # heads
D = 64            # head dim
HP = H // 2       # head pairs
SCALE = D ** -0.5


def _split_waits(nc):
    """Walrus in this toolchain encodes at most ONE sync wait per instruction;
    Tile emits several on drains. Hoist extras onto same-engine NoOps."""
    fn = nc.m.functions[0]
    n = 0
    for blk in fn.blocks:
        insts = blk.instructions
        i = 0
        while i < len(insts):
            inst = insts[i]
            si = getattr(inst, "sync_info", None)
            if si is not None and len(si.on_wait) > 1:
                waits = list(si.on_wait)
                pos = i
                for w in waits[:-1]:
                    nop = mybir.InstNoOp(name=nc.get_next_instruction_name(),
                                         ins=[], outs=[], engine=inst.engine)
                    nop.sync_info = mybir.SyncInfo(on_wait=[w], on_update=[])
                    insts.insert(pos, nop)
                    pos += 1
                    n += 1
                si.on_wait = waits[-1:]
                i = pos + 1
            else:
                i += 1
    return n


def _build():
    nc = bass.Bass()
    x_d = nc.dram_tensor("x", (N, C), F32, kind="ExternalInput")
    qkvw_d = nc.dram_tensor("qkv_w", (3 * C, C), F32, kind="ExternalInput")
    qkvb_d = nc.dram_tensor("qkv_b", (3 * C,), F32, kind="ExternalInput")
    projw_d = nc.dram_tensor("proj_w", (C, C), F32, kind="ExternalInput")
    projb_d = nc.dram_tensor("proj_b", (C,), F32, kind="ExternalInput")
    out_d = nc.dram_tensor("out", (N, C), F32, kind="ExternalOutput")
    attn_d = nc.dram_tensor("attn", (H, N, N), F32, kind="ExternalOutput")
    # scratch for turning per-partition 1/Z columns into free-dim rows
    scr_d = nc.dram_tensor("r_scratch", (HP, 2 * N), F32, kind="Internal")

    with tile.TileContext(nc) as tc:
        with tc.tile_pool(name="const", bufs=1) as const, \
             tc.tile_pool(name="persist", bufs=1) as persist:

            ident = const.tile([P, P], F32, tag="ident")
            make_identity(nc, ident)
            ones_f = const.tile([1, P], F32, tag="ones_f")
            nc.vector.memset(ones_f, 1.0)
            ones1 = const.tile([1, P], F32R, tag="ones1")
            nc.vector.tensor_copy(ones1, ones_f)

            qkvb_col = const.tile([P, 18], F32, tag="qkvb_col")
            nc.sync.dma_start(qkvb_col, qkvb_d.rearrange("(t p) -> p t", p=P))
            bias_stage = const.tile([1, 2 * C], F32, tag="bias_stage")
            nc.sync.dma_start(bias_stage[:, 0:C], qkvb_d[2 * C:3 * C][None])
            nc.sync.dma_start(bias_stage[:, C:2 * C], projb_d[None])
            qkvb_vr = const.tile([1, C], F32R, tag="qkvb_vr")
            nc.vector.tensor_copy(qkvb_vr, bias_stage[:, 0:C])
            projb_r = const.tile([1, C], F32R, tag="projb_r")
            nc.vector.tensor_copy(projb_r, bias_stage[:, C:2 * C])

            # persistent activations / weights
            qkT = [persist.tile([P, N], F32R, tag=f"qkT{t}", name=f"qkT{t}") for t in range(12)]
            V = [persist.tile([P, C], F32R, tag=f"V{j}", name=f"V{j}") for j in range(NT)]
            outT = [persist.tile([P, N], F32R, tag=f"outT{c}", name=f"outT{c}") for c in range(CT)]
            pwT = persist.tile([P, CT, C], F32R, tag="pwT")

            # ---------------- phase 0/1: transposes + QKV ----------------
            with tc.tile_pool(name="stage", bufs=2) as stage, \
                 tc.tile_pool(name="stage1", bufs=1) as stage1, \
                 tc.tile_pool(name="tp_ps", bufs=2, space="PSUM") as tp_ps, \
                 tc.tile_pool(name="mm_ps", bufs=2, space="PSUM") as mm_ps:

                xT = stage1.tile([P, CT, N], F32R, tag="xT")
                wTv = stage1.tile([P, CT, C], F32R, tag="wTv")

                def transpose_blocks(src, dst_fn):
                    # src: [128, 768] sbuf tile; transposes its six 128x128
                    # blocks; dst_fn(g, w) -> destination AP [P, w, 128]
                    for g, w in ((0, 4), (1, 2)):
                        tp = tp_ps.tile([P, 512], F32, tag="tp")
                        for k in range(w):
                            ct = g * 4 + k
                            nc.tensor.transpose(
                                tp[:, k * P:(k + 1) * P],
                                src[:, ct * P:(ct + 1) * P], ident)
                        nc.vector.tensor_copy(
                            dst_fn(g, w),
                            tp[:, :w * P].rearrange("p (k n) -> p k n", k=w))

                # x^T
                for nt in range(NT):
                    xa = stage.tile([P, C], F32, tag="xnat")
                    nc.sync.dma_start(xa, x_d[nt * P:(nt + 1) * P, :])
                    transpose_blocks(
                        xa, lambda g, w, nt=nt:
                        xT[:, g * 4:g * 4 + w, nt * P:(nt + 1) * P])

                # proj_w^T  (pwT[:, ct, o])
                for ot in range(CT):
                    wn = stage.tile([P, C], F32, tag="wnat")
                    nc.sync.dma_start(wn, projw_d[ot * P:(ot + 1) * P, :])
                    transpose_blocks(
                        wn, lambda g, w, ot=ot:
                        pwT[:, g * 4:g * 4 + w, ot * P:(ot + 1) * P])

                # V-part weights W_v^T (wTv[:, ct, o_v])
                for ov in range(CT):
                    wn = stage.tile([P, C], F32, tag="wnat")
                    nc.sync.dma_start(
                        wn, qkvw_d[2 * C + ov * P:2 * C + (ov + 1) * P, :])
                    transpose_blocks(
                        wn, lambda g, w, ov=ov:
                        wTv[:, g * 4:g * 4 + w, ov * P:(ov + 1) * P])

                # Q,K part: qkv^T layout, o-tile by o-tile
                for ot in range(12):
                    wn = stage.tile([P, C], F32, tag="wnat")
                    nc.sync.dma_start(wn, qkvw_d[ot * P:(ot + 1) * P, :])
                    wT = stage.tile([P, CT, P], F32R, tag="wT")
                    transpose_blocks(wn, lambda g, w: wT[:, g * 4:g * 4 + w, :])
                    ps = mm_ps.tile([P, N], F32, tag="mmps", name="qk_ps")
                    for ct in range(CT):
                        lhs = wT[:, ct, :]
                        nc.tensor.matmul(ps[:, 0:512], lhs, xT[:, ct, 0:512],
                                         start=(ct == 0), stop=(ct == CT - 1))
                        nc.tensor.matmul(ps[:, 512:1024], lhs,
                                         xT[:, ct, 512:1024],
                                         start=(ct == 0), stop=(ct == CT - 1))
                    # copyback + per-partition bias (o on partitions)
                    nc.vector.tensor_scalar_add(qkT[ot], ps,
                                                qkvb_col[:, ot:ot + 1])

                # V natural: V[nt] = x @ Wv^T + b_v
                for nt in range(NT):
                    ps = mm_ps.tile([P, N], F32, tag="mmps", name="v_ps")
                    for ct in range(CT):
                        lhs = xT[:, ct, nt * P:(nt + 1) * P]
                        nc.tensor.matmul(ps[:, 0:512], lhs, wTv[:, ct, 0:512],
                                         start=(ct == 0), stop=False)
                        nc.tensor.matmul(ps[:, 512:768], lhs,
                                         wTv[:, ct, 512:768],
                                         start=(ct == 0), stop=False)
                    nc.tensor.matmul(ps[:, 0:512], ones1, qkvb_vr[:, 0:512],
                                     start=False, stop=True)
                    nc.tensor.matmul(ps[:, 512:768], ones1, qkvb_vr[:, 512:768],
                                     start=False, stop=True)
                    nc.vector.tensor_copy(V[nt], ps[:, 0:C])

            # ---------------- phase 2: attention ----------------
            with tc.tile_pool(name="p_pool", bufs=9) as p_pool, \
                 tc.tile_pool(name="pT_pool", bufs=3) as pT_pool, \
                 tc.tile_pool(name="zr_pool", bufs=2) as zr_pool, \
                 tc.tile_pool(name="rr_pool", bufs=2) as rr_pool, \
                 tc.tile_pool(name="s_ps", bufs=2, space="PSUM") as s_ps, \
                 tc.tile_pool(name="o_ps", bufs=1, space="PSUM") as o_ps, \
                 tc.tile_pool(name="r_ps", bufs=1, space="PSUM") as r_ps:

                for hp in range(HP):
                    QT, KT = qkT[hp], qkT[6 + hp]
                    zcol = zr_pool.tile([P, 16], F32, tag="zcol")
                    rcol = zr_pool.tile([P, 16], F32, tag="rcol")

                    # --- S path: attn output + 1/Z stats ---
                    for h in (0, 1):
                        bp = D * h
                        hg = 2 * hp + h
                        p_tiles = []
                        for i in range(NT):
                            sps = s_ps.tile([P, N], F32, tag="sps")
                            lq = QT[bp:bp + D, i * P:(i + 1) * P]
                            nc.tensor.matmul(sps[:, 0:512], lq,
                                             KT[bp:bp + D, 0:512],
                                             start=True, stop=True)
                            nc.tensor.matmul(sps[:, 512:1024], lq,
                                             KT[bp:bp + D, 512:1024],
                                             start=True, stop=True)
                            pt = p_pool.tile([P, N], F32, tag="pt")
                            k = h * 8 + i
                            nc.scalar.activation(pt, sps, AF.Exp, scale=SCALE,
                                                 accum_out=zcol[:, k:k + 1])
                            p_tiles.append((pt, i, k))
                        nc.vector.reciprocal(rcol[:, h * 8:h * 8 + 8],
                                             zcol[:, h * 8:h * 8 + 8])
                        for pt, i, k in p_tiles:
                            nc.vector.tensor_scalar_mul(pt, pt,
                                                        rcol[:, k:k + 1])
                            nc.sync.dma_start(
                                attn_d[hg, i * P:(i + 1) * P, :], pt)

                    # 1/Z columns -> rows (free dim) via DRAM roundtrip
                    nc.sync.dma_start(
                        scr_d[hp].rearrange("(i p) -> p i", p=P), rcol)
                    rrow = {}
                    for h in (0, 1):
                        rf = rr_pool.tile([1, N], F32, tag="rrow_f")
                        nc.sync.dma_start(rf, scr_d[hp, h * N:(h + 1) * N][None])
                        rrow[h] = rf

                    # --- S^T path: P^T_raw and out^T accumulation ---
                    for h in (0, 1):
                        bp = D * h
                        hg = 2 * hp + h
                        ops = o_ps.tile([D, N], F32, tag="ops")
                        for j in range(NT):
                            sps = s_ps.tile([P, N], F32, tag="sps")
                            lk = KT[bp:bp + D, j * P:(j + 1) * P]
                            nc.tensor.matmul(sps[:, 0:512], lk,
                                             QT[bp:bp + D, 0:512],
                                             start=True, stop=True)
                            nc.tensor.matmul(sps[:, 512:1024], lk,
                                             QT[bp:bp + D, 512:1024],
                                             start=True, stop=True)
                            ptT = pT_pool.tile([P, N], F32R, tag="ptT")
                            nc.scalar.activation(ptT, sps, AF.Exp, scale=SCALE)
                            lv = V[j][:, hg * D:(hg + 1) * D]
                            nc.tensor.matmul(ops[:, 0:512], lv, ptT[:, 0:512],
                                             start=(j == 0), stop=(j == NT - 1),
                                             skip_group_check=True)
                            nc.tensor.matmul(ops[:, 512:1024], lv,
                                             ptT[:, 512:1024],
                                             start=(j == 0), stop=(j == NT - 1),
                                             skip_group_check=True)
                        # replicate 1/Z row across 64 partitions, fix up out^T
                        rps = r_ps.tile([D, N], F32, tag="rps")
                        nc.tensor.matmul(rps[:, 0:512], ones_f[:, 0:D],
                                         rrow[h][:, 0:512],
                                         start=True, stop=True)
                        nc.tensor.matmul(rps[:, 512:1024], ones_f[:, 0:D],
                                         rrow[h][:, 512:1024],
                                         start=True, stop=True)
                        rsb = rr_pool.tile([D, N], F32, tag="rsb")
                        nc.vector.tensor_copy(rsb, rps)
                        nc.vector.tensor_tensor(outT[hp][bp:bp + D, :],
                                                ops, rsb, MULT)

            # ---------------- phase 3: proj ----------------
            with tc.tile_pool(name="fsb_pool", bufs=3) as fsb_pool, \
                 tc.tile_pool(name="f_ps", bufs=2, space="PSUM") as f_ps:
                for nt in range(NT):
                    fps = f_ps.tile([P, C], F32, tag="fps")
                    for ct in range(CT):
                        lo = outT[ct][:, nt * P:(nt + 1) * P]
                        nc.tensor.matmul(fps[:, 0:512], lo, pwT[:, ct, 0:512],
                                         start=(ct == 0), stop=False)
                        nc.tensor.matmul(fps[:, 512:768], lo,
                                         pwT[:, ct, 512:768],
                                         start=(ct == 0), stop=False)
                    nc.tensor.matmul(fps[:, 0:512], ones1, projb_r[:, 0:512],
                                     start=False, stop=True)
                    nc.tensor.matmul(fps[:, 512:768], ones1,
                                     projb_r[:, 512:768],
                                     start=False, stop=True)
                    fsb = fsb_pool.tile([P, C], F32, tag="fsb")
                    nc.vector.tensor_copy(fsb, fps)
                    nc.sync.dma_start(out_d[nt * P:(nt + 1) * P, :], fsb)

    _split_waits(nc)
    return nc


_NC = None


def _get_nc():
    global _NC
    if _NC is None:
        _NC = _build()
    return _NC


def _run(inputs, trace=False, trace_kwargs=None):
    nc = _get_nc()
    x = np.ascontiguousarray(inputs["x"], dtype=np.float32)
    B = x.shape[0]
    shared = {k: np.ascontiguousarray(np.asarray(inputs[k], dtype=np.float32))
              for k in ("qkv_w", "qkv_b", "proj_w", "proj_b")}
    in_maps = [dict(shared, x=np.ascontiguousarray(x[b])) for b in range(B)]
    kw = {}
    if trace:
        kw = dict(trace=True, trace_cores=[0], **(trace_kwargs or {}))
    res = run_bass_kernel_spmd(nc, in_maps, core_ids=list(range(B)), **kw)
    out = np.stack([res.results[b]["out"] for b in range(B)])
    attn = np.stack([res.results[b]["attn"] for b in range(B)])
    return (out, attn), res


def kernel(**inputs):
    (out, attn), _ = _run(inputs)
    return (out, attn)
